# revision 24
# baseline (speedup 1.0000x reference)
"""Trainium2 Bass kernel for nn_CountingDiceLoss.

Reference math (B=8, H=W=512, P=40 centroids, 2-class dice + density-map MSE
+ squared count error):

  dm   = (sum_p exp(-((i-ci_p)^2+(j-cj_p)^2)/(2 s_k^2)) / (srpi*s_k))
         * bbox_mask / 2.50635
  p1   = softmax(x[:, :2])[:, 1] == sigmoid(x1 - x0)
  dc   = (2 tp + s) / (sum p1 + sum y + s)      (tp/fp/fn algebraic identity)
  loss = -mean_b(dc) + mean((x2 - dm)^2) + (sum x2 - sum dm)^2

Structure exploited:
  * The gaussian is separable: exp(-(di^2+dj^2)/2) = exp(-di^2/2)*exp(-dj^2/2),
    so the P-component accumulation is a rank-P outer-product sum — a
    [H,P] @ [P,W] TensorEngine matmul. The tiny 1-D factor tables
    (B*P*(H+W) elements, 0.3% of the input bytes) are precomputed on host
    with np.exp (also matches the reference's CPU f32 exp better than the
    ACT table, which has a ~1e-5 systematic bias).
  * Every reduction is fused into an elementwise pass it already needed
    (activation / scalar_tensor_tensor accum_out), finished in f64 on host.
    sum(x2) comes free via the identity sum(x2) = sum(x2-dm) + sum(dm);
    sum(y) runs on the otherwise-idle GpSimd engine.
  * One ~1MB dma_start per map with 8KB-contiguous runs (4 rows per
    partition) reaches HBM line rate; all DMAs share one FIFO HWDGE ring,
    so issue order = arrival order, chosen so each input's dependent chain
    overlaps the remaining stream (x2, the last input, is split in halves
    to pipeline its err->square tail).
  * When bbox_mask == y (true for the reference generator), one 1MB load
    is dropped and the y tile doubles as the mask (separate-variant
    fallback compiled on demand).

Sharding: data-parallel over batch; core c handles sample b=c (B == 8 cores).
"""

import numpy as np

import concourse.bacc as bacc
import concourse.bass as bass  # noqa: F401  (kept for users of this module)
import concourse.mybir as mybir
import concourse.tile as tile
from concourse.bass_utils import run_bass_kernel_spmd

B, H, W, P = 8, 512, 512, 40
NCORES = 8
RT = 128                 # partition tile
Q = H // RT              # 4 rows per partition (8KB contiguous DMA runs)
NSTAT = 9                # p1, dm_ab, sqerr_ab, err_ab, tp_ab

_sk = 2.0 ** (1.0 / 1e11)
_srpi = float(np.sqrt(2.0 * np.pi))
EXP_SCALE = float(-1.0 / (2.0 * _sk * _sk))      # ~ -0.5
POST = float(1.0 / (_srpi * _sk) / 2.50635)      # folded normalization

_F32 = mybir.dt.float32


def _emit(tc, nc, xc, yc, mc, g_d, stats_out, sy_out, shared_mask):
    A = mybir.AluOpType
    AF = mybir.ActivationFunctionType

    with (
        tc.tile_pool(name="const", bufs=1) as cpool,
        tc.tile_pool(name="inp", bufs=1) as ipool,
        tc.tile_pool(name="scr", bufs=1) as spool,
        tc.tile_pool(name="stat", bufs=1) as stpool,
        tc.tile_pool(name="psum", bufs=1, space="PSUM") as ppool,
    ):
        # ---- input DMAs (one FIFO ring: issue order == arrival order) ----
        gt = cpool.tile([P, 2, H], _F32)
        nc.sync.dma_start(gt[:], g_d.rearrange("t a j -> a t j"))
        gi, gj = gt[:, 0, :], gt[:, 1, :]

        def load_map(ap, tag):
            t = ipool.tile([RT, Q, W], _F32, tag=tag)
            nc.sync.dma_start(t[:], ap.rearrange("(p q) j -> p q j", p=RT))
            return t

        HQ = Q // 2

        def load_map_halves(ap, tag):
            t = ipool.tile([RT, Q, W], _F32, tag=tag)
            src = ap.rearrange("(p q) j -> p q j", p=RT)
            nc.sync.dma_start(t[:, 0:HQ], src[:, 0:HQ])
            nc.sync.dma_start(t[:, HQ:Q], src[:, HQ:Q])
            return t

        x0t = load_map(xc[0], "x0t")
        x1t = load_map(xc[1], "x1t")
        if shared_mask:
            # y doubles as the mask: split in halves so dmm starts earlier
            yt = load_map_halves(yc[:], "yt")
            mt = yt
        else:
            mt = load_map(mc[:], "mt")
            yt = load_map(yc[:], "yt")
        # x2 last, split in halves to pipeline its err->square tail
        x2t = load_map_halves(xc[2], "x2t")

        stats_sb = stpool.tile([RT, NSTAT], _F32)
        dmp = ppool.tile([RT, Q, W], _F32, tag="dmp")

        def col(s):
            return stats_sb[:, s:s + 1]

        # density map rows: partition p, free (q, j) holds row 4p+q
        gi_q = gi.rearrange("a (p q) -> a p q", q=Q)
        for q in range(Q):
            nc.tensor.matmul(
                dmp[:, q, :], gi_q[:, :, q], gj[:], start=True, stop=True,
            )

        # sum(y): exact integer partition sums on the otherwise-idle GpSimd
        sy_sb = stpool.tile([1, Q * W], _F32)
        nc.gpsimd.tensor_reduce(
            sy_sb[:], yt[:].rearrange("p a b -> p (a b)"),
            axis=mybir.AxisListType.C, op=A.add,
        )

        # p1 = sigmoid(x1 - x0); accum sum(p1)
        t01 = spool.tile([RT, Q, W], _F32)
        nc.vector.tensor_sub(t01[:], x1t[:], x0t[:])
        p1 = spool.tile([RT, Q, W], _F32)
        nc.scalar.activation(p1[:], t01[:], AF.Sigmoid, accum_out=col(0))

        halves = [(0, HQ), (HQ, Q)]

        # dm = (psum * POST) * mask; accum sum(dm). Halved: starts as soon
        # as the first y/mask half arrives.
        dmm = spool.tile([RT, Q, W], _F32)
        for h, (a, b) in enumerate(halves):
            nc.vector.scalar_tensor_tensor(
                dmm[:, a:b], dmp[:, a:b], POST, mt[:, a:b],
                op0=A.mult, op1=A.mult, accum_out=col(1 + h),
            )

        # err = x2 - dm with accum sum(err) [sum(x2) = sum(err) + sum(dm)];
        # squared+summed per half as the x2 halves arrive
        err = spool.tile([RT, Q, W], _F32)
        for h, (a, b) in enumerate(halves):
            nc.vector.scalar_tensor_tensor(
                err[:, a:b], x2t[:, a:b], 1.0, dmm[:, a:b],
                op0=A.mult, op1=A.subtract, accum_out=col(5 + h),
            )
            sq = spool.tile([RT, HQ, W], _F32, tag="sq")
            nc.scalar.activation(
                sq[:], err[:, a:b], AF.Square, accum_out=col(3 + h),
            )

        # tp partial: sum(p1 * y), halved
        prod = spool.tile([RT, Q, W], _F32)
        for h, (a, b) in enumerate(halves):
            nc.vector.scalar_tensor_tensor(
                prod[:, a:b], p1[:, a:b], 1.0, yt[:, a:b],
                op0=A.mult, op1=A.mult, accum_out=col(7 + h),
            )

        nc.sync.dma_start(stats_out[:], stats_sb[:])
        nc.sync.dma_start(sy_out[:], sy_sb[:])


_BUILT = {}


def _build(shared_mask):
    if shared_mask not in _BUILT:
        nc = bacc.Bacc(
            "TRN2", target_bir_lowering=False, debug=False, num_devices=NCORES,
        )
        xc = nc.dram_tensor("xc", [3, H, W], _F32, kind="ExternalInput").ap()
        yc = nc.dram_tensor("yc", [H, W], _F32, kind="ExternalInput").ap()
        mc = None
        if not shared_mask:
            mc = nc.dram_tensor("mc", [H, W], _F32, kind="ExternalInput").ap()
        g_d = nc.dram_tensor("g", [2, P, H], _F32, kind="ExternalInput").ap()
        stats = nc.dram_tensor(
            "stats", [RT, NSTAT], _F32, kind="ExternalOutput"
        ).ap()
        sy = nc.dram_tensor(
            "sy", [1, Q * W], _F32, kind="ExternalOutput"
        ).ap()
        with tile.TileContext(nc) as tc:
            _emit(tc, nc, xc, yc, mc, g_d, stats, sy, shared_mask)
        nc.compile()
        _BUILT[shared_mask] = nc
    return _BUILT[shared_mask]


def make_in_maps(x, y, bbox_mask, centroids, valid, shared_mask):
    x = np.ascontiguousarray(np.asarray(x, dtype=np.float32))
    y = np.ascontiguousarray(np.asarray(y, dtype=np.float32))
    bbox_mask = np.ascontiguousarray(np.asarray(bbox_mask, dtype=np.float32))
    centroids = np.asarray(centroids)
    validf = np.asarray(valid).astype(np.float32)

    # 1-D gaussian factor tables (separable kernel), f32 like the reference
    idx = np.arange(H, dtype=np.float32)
    ci = centroids[..., 0].astype(np.float32)[..., None]   # [B,P,1]
    cj = centroids[..., 1].astype(np.float32)[..., None]
    gi = np.exp(((idx[None, None, :] - ci) ** 2) * np.float32(EXP_SCALE))
    gi = gi * validf[..., None]
    gj = np.exp(((idx[None, None, :] - cj) ** 2) * np.float32(EXP_SCALE))
    g = np.ascontiguousarray(np.stack([gi, gj], axis=1).astype(np.float32))

    maps = []
    for c in range(NCORES):
        m = {"xc": x[c], "yc": y[c, 0], "g": g[c]}
        if not shared_mask:
            m["mc"] = bbox_mask[c, 0]
        maps.append(m)
    return maps


def combine(results):
    """results: per-core dicts with stats [128, NSTAT] -> scalar loss."""
    s = np.stack(
        [r["stats"].astype(np.float64).sum(axis=0) for r in results]
    )  # [B, NSTAT]
    sum_p1 = s[:, 0]
    sum_dm = s[:, 1] + s[:, 2]
    sum_sq = s[:, 3] + s[:, 4]
    sum_x2 = s[:, 5] + s[:, 6] + sum_dm
    tp = s[:, 7] + s[:, 8]
    sum_y = np.array(
        [r["sy"].astype(np.float64).sum() for r in results]
    )
    smooth = 1e-5
    dc = (2.0 * tp + smooth) / (sum_p1 + sum_y + smooth)
    l_dice = -dc.mean()
    l_dm = sum_sq.sum() / (B * H * W)
    l_n = (sum_x2.sum() - sum_dm.sum()) ** 2
    return np.float32(l_dice + l_dm + l_n)


LAST_RESULT = None  # BassKernelResults of the most recent run (for profiling)


def kernel(x, y, bbox_mask, centroids, valid):
    global LAST_RESULT
    shared = np.array_equal(
        np.asarray(y, dtype=np.float32), np.asarray(bbox_mask, dtype=np.float32)
    )
    nc = _build(shared)
    in_maps = make_in_maps(x, y, bbox_mask, centroids, valid, shared)
    res = run_bass_kernel_spmd(nc, in_maps, list(range(NCORES)))
    LAST_RESULT = res
    return combine(res.results)


# revision 26
# speedup vs baseline: 9.0555x; 9.0555x over previous
"""Trainium2 Bass kernel for nn_CountingDiceLoss.

Reference math (B=8, H=W=512, P=40 centroids, 2-class dice + density-map MSE
+ squared count error):

  dm   = (sum_p exp(-((i-ci_p)^2+(j-cj_p)^2)/(2 s_k^2)) / (srpi*s_k))
         * bbox_mask / 2.50635
  p1   = softmax(x[:, :2])[:, 1] == sigmoid(x1 - x0)
  dc   = (2 tp + s) / (sum p1 + sum y + s)      (tp/fp/fn algebraic identity)
  loss = -mean_b(dc) + mean((x2 - dm)^2) + (sum x2 - sum dm)^2

Structure exploited:
  * The gaussian is separable: exp(-(di^2+dj^2)/2) = exp(-di^2/2)*exp(-dj^2/2),
    so the P-component accumulation is a rank-P outer-product sum — a
    [H,P] @ [P,W] TensorEngine matmul. The tiny 1-D factor tables
    (B*P*(H+W) elements, 0.3% of the input bytes) are precomputed on host
    with np.exp (also matches the reference's CPU f32 exp better than the
    ACT table, which has a ~1e-5 systematic bias).
  * Every reduction is fused into an elementwise pass it already needed
    (activation / scalar_tensor_tensor accum_out), finished in f64 on host.
    sum(x2) comes free via the identity sum(x2) = sum(x2-dm) + sum(dm);
    sum(y) runs on the otherwise-idle GpSimd engine.
  * One ~1MB dma_start per map with 8KB-contiguous runs (4 rows per
    partition) reaches HBM line rate; all DMAs share one FIFO HWDGE ring,
    so issue order = arrival order, chosen so each input's dependent chain
    overlaps the remaining stream (x2, the last input, is split in halves
    to pipeline its err->square tail).
  * When bbox_mask == y (true for the reference generator), one 1MB load
    is dropped and the y tile doubles as the mask (separate-variant
    fallback compiled on demand).

Sharding: data-parallel over batch; core c handles sample b=c (B == 8 cores).
"""

import numpy as np

import concourse.bacc as bacc
import concourse.bass as bass  # noqa: F401  (kept for users of this module)
import concourse.mybir as mybir
import concourse.tile as tile
from concourse.bass_utils import run_bass_kernel_spmd

B, H, W, P = 8, 512, 512, 40
NCORES = 8
RT = 128                 # partition tile
Q = H // RT              # 4 rows per partition (8KB contiguous DMA runs)
NSTAT = 9                # p1, dm_ab, sqerr_ab, err_ab, tp_ab

_sk = 2.0 ** (1.0 / 1e11)
_srpi = float(np.sqrt(2.0 * np.pi))
EXP_SCALE = float(-1.0 / (2.0 * _sk * _sk))      # ~ -0.5
POST = float(1.0 / (_srpi * _sk) / 2.50635)      # folded normalization

_F32 = mybir.dt.float32


def _emit(tc, nc, xc, yc, mc, g_d, stats_out, sy_out, shared_mask):
    A = mybir.AluOpType
    AF = mybir.ActivationFunctionType

    with (
        tc.tile_pool(name="const", bufs=1) as cpool,
        tc.tile_pool(name="inp", bufs=1) as ipool,
        tc.tile_pool(name="scr", bufs=1) as spool,
        tc.tile_pool(name="stat", bufs=1) as stpool,
        tc.tile_pool(name="psum", bufs=1, space="PSUM") as ppool,
    ):
        # ---- input DMAs (one FIFO ring: issue order == arrival order) ----
        gt = cpool.tile([P, 2, H], _F32)
        nc.sync.dma_start(gt[:], g_d.rearrange("t a j -> a t j"))
        gi, gj = gt[:, 0, :], gt[:, 1, :]

        def load_map(ap, tag):
            t = ipool.tile([RT, Q, W], _F32, tag=tag)
            nc.sync.dma_start(t[:], ap.rearrange("(p q) j -> p q j", p=RT))
            return t

        HQ = Q // 2

        def load_map_halves(ap, tag):
            t = ipool.tile([RT, Q, W], _F32, tag=tag)
            src = ap.rearrange("(p q) j -> p q j", p=RT)
            nc.sync.dma_start(t[:, 0:HQ], src[:, 0:HQ])
            nc.sync.dma_start(t[:, HQ:Q], src[:, HQ:Q])
            return t

        x0t = load_map(xc[0], "x0t")
        x1t = load_map(xc[1], "x1t")
        if shared_mask:
            # y doubles as the mask: split in halves so dmm starts earlier
            yt = load_map_halves(yc[:], "yt")
            mt = yt
        else:
            mt = load_map(mc[:], "mt")
            yt = load_map(yc[:], "yt")
        # x2 last, split in halves to pipeline its err->square tail
        x2t = load_map_halves(xc[2], "x2t")

        stats_sb = stpool.tile([RT, NSTAT], _F32)
        dmp = ppool.tile([RT, Q, W], _F32, tag="dmp")

        def col(s):
            return stats_sb[:, s:s + 1]

        # density map rows: partition p, free (q, j) holds row 4p+q
        gi_q = gi.rearrange("a (p q) -> a p q", q=Q)
        for q in range(Q):
            nc.tensor.matmul(
                dmp[:, q, :], gi_q[:, :, q], gj[:], start=True, stop=True,
            )

        # sum(y): exact integer column sums via PE ones-matmul (PE is idle
        # once the 4 density-map matmuls finish)
        ones = cpool.tile([RT, 1], _F32)
        nc.gpsimd.memset(ones[:], 1.0)
        sy_ps = ppool.tile([1, W], _F32, tag="sy_ps")
        for q in range(Q):
            nc.tensor.matmul(
                sy_ps[:], ones[:, 0:1], yt[:, q, :],
                start=q == 0, stop=q == Q - 1, skip_group_check=True,
            )
        sy_sb = stpool.tile([1, W], _F32)
        nc.scalar.copy(sy_sb[:], sy_ps[:])

        # p1 = sigmoid(x1 - x0); accum sum(p1)
        t01 = spool.tile([RT, Q, W], _F32)
        nc.vector.tensor_sub(t01[:], x1t[:], x0t[:])
        p1 = spool.tile([RT, Q, W], _F32)
        nc.scalar.activation(p1[:], t01[:], AF.Sigmoid, accum_out=col(0))

        halves = [(0, HQ), (HQ, Q)]

        # dm = (psum * POST) * mask; accum sum(dm). Halved: starts as soon
        # as the first y/mask half arrives.
        dmm = spool.tile([RT, Q, W], _F32)
        for h, (a, b) in enumerate(halves):
            nc.vector.scalar_tensor_tensor(
                dmm[:, a:b], dmp[:, a:b], POST, mt[:, a:b],
                op0=A.mult, op1=A.mult, accum_out=col(1 + h),
            )

        # err = x2 - dm with accum sum(err) [sum(x2) = sum(err) + sum(dm)];
        # squared+summed per half as the x2 halves arrive
        err = spool.tile([RT, Q, W], _F32)
        for h, (a, b) in enumerate(halves):
            nc.vector.scalar_tensor_tensor(
                err[:, a:b], x2t[:, a:b], 1.0, dmm[:, a:b],
                op0=A.mult, op1=A.subtract, accum_out=col(5 + h),
            )
            sq = spool.tile([RT, HQ, W], _F32, tag="sq")
            nc.scalar.activation(
                sq[:], err[:, a:b], AF.Square, accum_out=col(3 + h),
            )

        # tp partial: sum(p1 * y), halved
        prod = spool.tile([RT, Q, W], _F32)
        for h, (a, b) in enumerate(halves):
            nc.vector.scalar_tensor_tensor(
                prod[:, a:b], p1[:, a:b], 1.0, yt[:, a:b],
                op0=A.mult, op1=A.mult, accum_out=col(7 + h),
            )

        nc.sync.dma_start(stats_out[:], stats_sb[:])
        nc.sync.dma_start(sy_out[:], sy_sb[:])


_BUILT = {}


def _build(shared_mask):
    if shared_mask not in _BUILT:
        nc = bacc.Bacc(
            "TRN2", target_bir_lowering=False, debug=False, num_devices=NCORES,
        )
        xc = nc.dram_tensor("xc", [3, H, W], _F32, kind="ExternalInput").ap()
        yc = nc.dram_tensor("yc", [H, W], _F32, kind="ExternalInput").ap()
        mc = None
        if not shared_mask:
            mc = nc.dram_tensor("mc", [H, W], _F32, kind="ExternalInput").ap()
        g_d = nc.dram_tensor("g", [2, P, H], _F32, kind="ExternalInput").ap()
        stats = nc.dram_tensor(
            "stats", [RT, NSTAT], _F32, kind="ExternalOutput"
        ).ap()
        sy = nc.dram_tensor("sy", [1, W], _F32, kind="ExternalOutput").ap()
        with tile.TileContext(nc) as tc:
            _emit(tc, nc, xc, yc, mc, g_d, stats, sy, shared_mask)
        nc.compile()
        _BUILT[shared_mask] = nc
    return _BUILT[shared_mask]


def make_in_maps(x, y, bbox_mask, centroids, valid, shared_mask):
    x = np.ascontiguousarray(np.asarray(x, dtype=np.float32))
    y = np.ascontiguousarray(np.asarray(y, dtype=np.float32))
    bbox_mask = np.ascontiguousarray(np.asarray(bbox_mask, dtype=np.float32))
    centroids = np.asarray(centroids)
    validf = np.asarray(valid).astype(np.float32)

    # 1-D gaussian factor tables (separable kernel), f32 like the reference
    idx = np.arange(H, dtype=np.float32)
    ci = centroids[..., 0].astype(np.float32)[..., None]   # [B,P,1]
    cj = centroids[..., 1].astype(np.float32)[..., None]
    gi = np.exp(((idx[None, None, :] - ci) ** 2) * np.float32(EXP_SCALE))
    gi = gi * validf[..., None]
    gj = np.exp(((idx[None, None, :] - cj) ** 2) * np.float32(EXP_SCALE))
    g = np.ascontiguousarray(np.stack([gi, gj], axis=1).astype(np.float32))

    maps = []
    for c in range(NCORES):
        m = {"xc": x[c], "yc": y[c, 0], "g": g[c]}
        if not shared_mask:
            m["mc"] = bbox_mask[c, 0]
        maps.append(m)
    return maps


def combine(results):
    """results: per-core dicts with stats [128, NSTAT] -> scalar loss."""
    s = np.stack(
        [r["stats"].astype(np.float64).sum(axis=0) for r in results]
    )  # [B, NSTAT]
    sum_p1 = s[:, 0]
    sum_dm = s[:, 1] + s[:, 2]
    sum_sq = s[:, 3] + s[:, 4]
    sum_x2 = s[:, 5] + s[:, 6] + sum_dm
    tp = s[:, 7] + s[:, 8]
    sum_y = np.array(
        [r["sy"].astype(np.float64).sum() for r in results]
    )
    smooth = 1e-5
    dc = (2.0 * tp + smooth) / (sum_p1 + sum_y + smooth)
    l_dice = -dc.mean()
    l_dm = sum_sq.sum() / (B * H * W)
    l_n = (sum_x2.sum() - sum_dm.sum()) ** 2
    return np.float32(l_dice + l_dm + l_n)


LAST_RESULT = None  # BassKernelResults of the most recent run (for profiling)


def kernel(x, y, bbox_mask, centroids, valid):
    global LAST_RESULT
    shared = np.array_equal(
        np.asarray(y, dtype=np.float32), np.asarray(bbox_mask, dtype=np.float32)
    )
    nc = _build(shared)
    in_maps = make_in_maps(x, y, bbox_mask, centroids, valid, shared)
    res = run_bass_kernel_spmd(nc, in_maps, list(range(NCORES)))
    LAST_RESULT = res
    return combine(res.results)
